# revision 1
# baseline (speedup 1.0000x reference)
"""Trainium2 Bass kernel for 3D deformable attention (8 NeuronCores).

Sharding: core i handles batch b = i // 4 and head-pair j = i % 4
(heads 2j, 2j+1, i.e. value/out channels [64j, 64j+64)).

Per-core device pipeline:
  1. value projection  v = W_val[64j:64j+64] @ value[b]  (PE, voxel-stationary
     so the output lands voxel-major), + b_val, stored to DRAM as two
     head-blocks of [NVOX+8, 32] (VDT dtype).
  2. coords = qs^T @ Wcat^T + [48*ref | ones*bias]  (PE) giving, per query,
     30 pixel coords (2 heads x 5 pts x 3 axes, biased +64) and 10 logits.
  3. DVE/ACT: softmax over 5 points, clamped trilinear corner weights for
     x/y, z handled as a W-slot "hat" over a 4-voxel-aligned window, int16
     gather row indices, combined mask m[r, k] = aw*wx*wy*wz[k].
  4. idx shuffle via DRAM round trip into dma_gather's 16-partition-wrapped
     layout; per (query-subtile, head) one dma_gather of 2560 rows
     (W voxels x 32ch each).
  5. DVE: P = G * mask (broadcast over channel), strided-AP reduce over
     (p,dx,dy,k) -> S[q, 64].
  6. PE transpose of S, then out^T = Wout_cols^T @ S^T, DMA out.
Host combines: out[b] = sum_j outp_j^T + b_out.
"""
import numpy as np

import concourse.bass as bass
import concourse.mybir as mybir
from concourse import bacc, tile
from concourse.masks import make_identity
from contextlib import ExitStack

F32 = mybir.dt.float32
I16 = mybir.dt.int16
AX = mybir.AxisListType
OP = mybir.AluOpType
ACT = mybir.ActivationFunctionType

H, P = 8, 5
NQ, C, GRID = 4096, 256, 48
NVOX = GRID ** 3            # 110592
NSUB = 4                    # query subtiles (of 128) per supertile
TQ = 128 * NSUB             # 512
NSUP = NQ // TQ             # 8
VSUP = 1024                 # voxels per value-proj supertile
NVSUP = NVOX // VSUP        # 108
NR = 4                      # value supertiles per DRAM flush

# gather configuration
GATHER_FP32 = False
VDT = F32 if GATHER_FP32 else mybir.dt.bfloat16
W = 6 if GATHER_FP32 else 8         # voxels per gathered row
SROW = 4                            # voxels per row step (4*32*VDT bytes % 256 == 0)
NROWH = NVOX // SROW                # 27648 rows per head (< 32768 for int16)
BSV = NVOX + 8                      # voxels per head block incl pad
NIDX = 20 * 128                     # rows per (subtile, head) gather

DEBUG = False

_NC_CACHE = None


def build_nc():
    nc = bacc.Bacc("TRN2", target_bir_lowering=False, debug=False, num_devices=8)

    value_in = nc.dram_tensor("value_in", [C, NVOX], F32, kind="ExternalInput")
    qT = nc.dram_tensor("qT", [C, NQ], F32, kind="ExternalInput")
    pT = nc.dram_tensor("pT", [C, NQ], F32, kind="ExternalInput")
    refT = nc.dram_tensor("refT", [4, NQ], F32, kind="ExternalInput")
    wcat = nc.dram_tensor("wcat", [C, 40], F32, kind="ExternalInput")
    ref_rhs = nc.dram_tensor("ref_rhs", [4, 40], F32, kind="ExternalInput")
    wval = nc.dram_tensor("wval", [C, 64], F32, kind="ExternalInput")
    bval = nc.dram_tensor("bval", [128, 64], F32, kind="ExternalInput")
    wout = nc.dram_tensor("wout", [64, C], F32, kind="ExternalInput")
    zoff = nc.dram_tensor("zoff", [128, W], F32, kind="ExternalInput")
    outp = nc.dram_tensor("outp", [C, NQ], F32, kind="ExternalOutput")
    vflat = nc.dram_tensor("vflat", [2 * BSV * 32], VDT)
    idxscr = nc.dram_tensor("idxscr", [NSUP * 128, 160], I16)

    dbg = {}
    if DEBUG:
        for nm, sh in [("d_coords", [128, 160]), ("d_mask", [128, NSUB * 40 * W]),
                       ("d_idxf", [128, 160]), ("d_S", [128, 64])]:
            dbg[nm] = nc.dram_tensor(nm, sh, F32, kind="ExternalOutput")
        dbg["d_G"] = nc.dram_tensor("d_G", [128, 20 * W * 32], VDT,
                                    kind="ExternalOutput")

    vec = nc.vector
    act = nc.scalar

    with tile.TileContext(nc) as tc, ExitStack() as ctx:
        const = ctx.enter_context(tc.tile_pool(name="const", bufs=1))
        vpool = ctx.enter_context(tc.tile_pool(name="vpool", bufs=2))
        qpool = ctx.enter_context(tc.tile_pool(name="qpool", bufs=2))
        gpool = ctx.enter_context(tc.tile_pool(name="gpool", bufs=2))
        opool = ctx.enter_context(tc.tile_pool(name="opool", bufs=2))
        ps_v = ctx.enter_context(tc.tile_pool(name="ps_v", bufs=2, space="PSUM"))
        ps_c = ctx.enter_context(tc.tile_pool(name="ps_c", bufs=2, space="PSUM"))
        ps_t = ctx.enter_context(tc.tile_pool(name="ps_t", bufs=2, space="PSUM"))

        # ---- constants into SBUF ----
        wcat_sb = [const.tile([128, 40], F32, tag=f"wcat{k}", name=f"wcat{k}")
                   for k in range(2)]
        for k in range(2):
            nc.sync.dma_start(out=wcat_sb[k][:], in_=wcat[k * 128:(k + 1) * 128, :])
        refrhs_sb = const.tile([4, 40], F32, tag="refrhs", name="refrhs")
        nc.sync.dma_start(out=refrhs_sb[:], in_=ref_rhs[:])
        wval_sb = [const.tile([128, 64], F32, tag=f"wval{k}", name=f"wval{k}")
                   for k in range(2)]
        for k in range(2):
            nc.sync.dma_start(out=wval_sb[k][:], in_=wval[k * 128:(k + 1) * 128, :])
        bval_sb = const.tile([128, 64], F32, tag="bval", name="bval")
        nc.sync.dma_start(out=bval_sb[:], in_=bval[:])
        wout_sb = const.tile([64, C], F32, tag="wout", name="wout")
        nc.sync.dma_start(out=wout_sb[:], in_=wout[:])
        zoff_sb = const.tile([128, W], F32, tag="zoff", name="zoff")
        nc.sync.dma_start(out=zoff_sb[:], in_=zoff[:])
        ident = const.tile([128, 128], F32, tag="ident", name="ident")
        make_identity(nc, ident[:])

        # persistent big buffers
        qs_sb = [const.tile([128, NQ], F32, tag=f"qs{k}", name=f"qs{k}")
                 for k in range(2)]
        ref_sb = const.tile([4, NQ], F32, tag="refq", name="refq")
        st_sb = const.tile([64, NQ], F32, tag="st", name="st")

        # ---- stage Q0: load q, pos, ref; qs = q + p ----
        for k in range(2):
            for half in range(2):
                sl = slice(half * (NQ // 2), (half + 1) * (NQ // 2))
                ptmp = qpool.tile([128, NQ // 2], F32, tag="ptmp", name="ptmp")
                nc.sync.dma_start(out=qs_sb[k][:, sl],
                                  in_=qT[k * 128:(k + 1) * 128, sl])
                nc.sync.dma_start(out=ptmp[:], in_=pT[k * 128:(k + 1) * 128, sl])
                vec.tensor_tensor(out=qs_sb[k][:, sl], in0=qs_sb[k][:, sl],
                                  in1=ptmp[:], op=OP.add)
        nc.sync.dma_start(out=ref_sb[:], in_=refT[:])

        # ---- stage V: value projection ----
        # zero the pad voxels at the end of each head block
        zpad = const.tile([8, 32], VDT, tag="zpad", name="zpad")
        vec.memset(zpad[:], 0.0)
        vflat_r = vflat[:].rearrange("(v c) -> v c", c=32)
        for hl in range(2):
            nc.sync.dma_start(
                out=vflat_r[hl * BSV + NVOX:hl * BSV + NVOX + 8, :], in_=zpad[:])
        for vg in range(NVSUP // NR):          # flush groups of NR supertiles
            vb = [vpool.tile([128, NR * 256], VDT, tag=f"vb{hl}", name=f"vb{hl}")
                  for hl in range(2)]
            for i in range(NR):
                vt = vg * NR + i
                vin = [vpool.tile([128, VSUP], F32, tag=f"vin{k}", name=f"vin{k}")
                       for k in range(2)]
                for k in range(2):
                    nc.sync.dma_start(
                        out=vin[k][:],
                        in_=value_in[k * 128:(k + 1) * 128,
                                     vt * VSUP:(vt + 1) * VSUP])
                psv = ps_v.tile([128, 512], F32, tag="psv", name="psv")
                for s in range(8):
                    lhs0 = vin[0][:].rearrange("p (v e) -> p e v", e=8)[:, s, :]
                    lhs1 = vin[1][:].rearrange("p (v e) -> p e v", e=8)[:, s, :]
                    nc.tensor.matmul(psv[:, s * 64:(s + 1) * 64], lhs0,
                                     wval_sb[0][:], start=True, stop=False)
                    nc.tensor.matmul(psv[:, s * 64:(s + 1) * 64], lhs1,
                                     wval_sb[1][:], start=False, stop=True)
                # split heads, add bias, pack [128, (s,c)=256] per head
                psr = psv[:].rearrange("p (s hc) -> p s hc", s=8)
                for hl in range(2):
                    bv = bval_sb[:, hl * 32:(hl + 1) * 32] \
                        .unsqueeze(1).to_broadcast([128, 8, 32])
                    vec.tensor_tensor(
                        out=vb[hl][:, i * 256:(i + 1) * 256]
                            .rearrange("p (s c) -> p s c", s=8),
                        in0=psr[:, :, hl * 32:(hl + 1) * 32],
                        in1=bv, op=OP.add)
            # flush NR supertiles (NR*1024 voxels) per head
            for hl in range(2):
                base = hl * BSV + vg * NR * VSUP
                dst = vflat_r[base:base + NR * VSUP, :] \
                    .rearrange("(i p s) c -> p i (s c)", i=NR, p=128)
                nc.sync.dma_start(out=dst, in_=vb[hl][:]
                                  .rearrange("p (i sc) -> p i sc", i=NR))

        # ---- stage Q: per supertile of 512 queries ----
        for g in range(NSUP):
            q0 = g * TQ
            psc = ps_c.tile([128, 160], F32, tag="psc", name="psc")
            for s in range(NSUB):
                qsl = slice(q0 + s * 128, q0 + (s + 1) * 128)
                nc.tensor.matmul(psc[:, s * 40:(s + 1) * 40],
                                 qs_sb[0][:, qsl], wcat_sb[0][:],
                                 start=True, stop=False)
                nc.tensor.matmul(psc[:, s * 40:(s + 1) * 40],
                                 qs_sb[1][:, qsl], wcat_sb[1][:],
                                 start=False, stop=False)
                nc.tensor.matmul(psc[:, s * 40:(s + 1) * 40],
                                 ref_sb[:, qsl], refrhs_sb[:],
                                 start=False, stop=True)
            coords = qpool.tile([128, 160], F32, tag="coords", name="coords")
            act.activation(out=coords[:], in_=psc[:], func=ACT.Copy)
            if DEBUG and g == 0:
                nc.sync.dma_start(out=dbg["d_coords"][:], in_=coords[:])

            co = coords[:].rearrange("p (s r) -> p s r", s=NSUB)
            pix = co[:, :, 0:30]                        # (s, hp*ax)
            logit = co[:, :, 30:40]                     # (s, hp)

            # softmax over P
            exlog = qpool.tile([128, NSUB * 10], F32, tag="exlog", name="exlog")
            act.activation(out=exlog[:], in_=logit, func=ACT.Exp)
            ex4 = exlog[:].rearrange("p (s h q) -> p s h q", s=NSUB, h=2)
            sums = qpool.tile([128, NSUB * 2], F32, tag="sums", name="sums")
            vec.tensor_reduce(out=sums[:].rearrange("p (s h) -> p s h", s=NSUB),
                              in_=ex4, axis=AX.X, op=OP.add)
            rsum = qpool.tile([128, NSUB * 2], F32, tag="rsum", name="rsum")
            vec.reciprocal(out=rsum[:], in_=sums[:])
            aw = qpool.tile([128, NSUB * 10], F32, tag="aw", name="aw")
            vec.tensor_tensor(
                out=aw[:].rearrange("p (sh q) -> p sh q", q=5),
                in0=exlog[:].rearrange("p (sh q) -> p sh q", q=5),
                in1=rsum[:].unsqueeze(2).to_broadcast([128, NSUB * 2, 5]),
                op=OP.mult)

            # corner math on the 30 pixel rows
            NPX = NSUB * 30
            # flo = round(pix - 0.5) via the 2^23 magic add (== floor except
            # exactly-integer pix, where the phantom corner gets zero weight)
            flo = qpool.tile([128, NPX], F32, tag="flo", name="flo")
            vec.tensor_scalar(out=flo[:].rearrange("p (s r) -> p s r", s=NSUB),
                              in0=pix, scalar1=8388607.5, scalar2=8388608.0,
                              op0=OP.add, op1=OP.subtract)
            fl3 = flo[:].rearrange("p (s r) -> p s r", s=NSUB)
            frac = qpool.tile([128, NPX], F32, tag="frac", name="frac")
            vec.tensor_tensor(out=frac[:].rearrange("p (s r) -> p s r", s=NSUB),
                              in0=pix, in1=fl3, op=OP.subtract)
            fr3 = frac[:].rearrange("p (s r) -> p s r", s=NSUB)
            # gcorn: (s, hp, ax, dx) — clamped corner coords (biased +64)
            gcorn = qpool.tile([128, NPX * 2], F32, tag="gcorn", name="gcorn")
            gc4 = gcorn[:].rearrange("p (s r d) -> p s r d", s=NSUB, d=2)
            vec.tensor_scalar(out=gc4[:, :, :, 0], in0=fl3,
                              scalar1=64.0, scalar2=111.0, op0=OP.max, op1=OP.min)
            g1m = qpool.tile([128, NPX], F32, tag="g1m", name="g1m")
            vec.tensor_scalar(out=g1m[:], in0=flo[:],
                              scalar1=63.0, scalar2=110.0, op0=OP.max, op1=OP.min)
            vec.tensor_scalar(out=gc4[:, :, :, 1], in0=g1m[:]
                              .rearrange("p (s r) -> p s r", s=NSUB),
                              scalar1=1.0, scalar2=None, op0=OP.add)
            # validity via clip-equality
            v0 = qpool.tile([128, NPX], F32, tag="v0", name="v0")
            vec.tensor_tensor(out=v0[:].rearrange("p (s r) -> p s r", s=NSUB),
                              in0=gc4[:, :, :, 0], in1=fl3, op=OP.is_equal)
            v1 = qpool.tile([128, NPX], F32, tag="v1", name="v1")
            vec.tensor_tensor(out=v1[:], in0=g1m[:], in1=flo[:], op=OP.is_equal)
            # corner weights (x/y rows used; z rows ignored later)
            om = qpool.tile([128, NPX], F32, tag="om", name="om")
            vec.tensor_scalar(out=om[:], in0=frac[:], scalar1=-1.0, scalar2=1.0,
                              op0=OP.mult, op1=OP.add)
            wcorn = qpool.tile([128, NPX * 2], F32, tag="wcorn", name="wcorn")
            wc4 = wcorn[:].rearrange("p (s r d) -> p s r d", s=NSUB, d=2)
            vec.tensor_tensor(out=wc4[:, :, :, 0],
                              in0=om[:].rearrange("p (s r) -> p s r", s=NSUB),
                              in1=v0[:].rearrange("p (s r) -> p s r", s=NSUB),
                              op=OP.mult)
            vec.tensor_tensor(out=wc4[:, :, :, 1],
                              in0=fr3,
                              in1=v1[:].rearrange("p (s r) -> p s r", s=NSUB),
                              op=OP.mult)

            # z window: rzq = floor((gz-64)/4) in [0,11]; W-slot hat weights
            gc6 = gcorn[:].rearrange("p (s hp a d) -> p s hp a d",
                                     s=NSUB, hp=10, a=3)
            gz = gc6[:, :, :, 2, 0]
            pz = co[:, :, 0:30].rearrange("p s (hp a) -> p s hp a", a=3)[:, :, :, 2]
            rzq = qpool.tile([128, NSUB * 10], F32, tag="rzq", name="rzq")
            tq = qpool.tile([128, NSUB * 10], F32, tag="tq", name="tq")
            vec.tensor_scalar(out=tq[:].rearrange("p (s h) -> p s h", s=NSUB),
                              in0=gz, scalar1=0.25, scalar2=16.375,
                              op0=OP.mult, op1=OP.subtract)
            vec.tensor_scalar(out=rzq[:], in0=tq[:],
                              scalar1=8388624.0, scalar2=8388624.0,
                              op0=OP.add, op1=OP.subtract)
            # d0 = (4*rzq + 64) - pz ; dk = d0 + k
            zb4 = qpool.tile([128, NSUB * 10], F32, tag="zb4", name="zb4")
            vec.tensor_scalar(out=zb4[:], in0=rzq[:], scalar1=4.0, scalar2=64.0,
                              op0=OP.mult, op1=OP.add)
            d0 = qpool.tile([128, NSUB * 10], F32, tag="d0", name="d0")
            vec.tensor_tensor(out=d0[:].rearrange("p (s h) -> p s h", s=NSUB),
                              in0=zb4[:].rearrange("p (s h) -> p s h", s=NSUB),
                              in1=pz, op=OP.subtract)
            dk = qpool.tile([128, NSUB * 10 * W], F32, tag="dk", name="dk")
            vec.tensor_tensor(
                out=dk[:].rearrange("p (sh k) -> p sh k", k=W),
                in0=d0[:].unsqueeze(2).to_broadcast([128, NSUB * 10, W]),
                in1=zoff_sb[:].unsqueeze(1).to_broadcast([128, NSUB * 10, W]),
                op=OP.add)
            adk = qpool.tile([128, NSUB * 10 * W], F32, tag="adk", name="adk")
            act.activation(out=adk[:], in_=dk[:], func=ACT.Abs)
            hat = qpool.tile([128, NSUB * 10 * W], F32, tag="hat", name="hat")
            act.activation(out=hat[:], in_=adk[:], func=ACT.Relu,
                           scale=-1.0, bias=1.0)
            # upper bound: slot z 4*rzq+64+k <= 111  <=>  dk <= 111 - pz
            ub = qpool.tile([128, NSUB * 10], F32, tag="ub", name="ub")
            vec.tensor_scalar(out=ub[:].rearrange("p (s h) -> p s h", s=NSUB),
                              in0=pz, scalar1=-1.0, scalar2=111.0,
                              op0=OP.mult, op1=OP.add)
            vub = qpool.tile([128, NSUB * 10 * W], F32, tag="vub", name="vub")
            vec.tensor_tensor(
                out=vub[:].rearrange("p (sh k) -> p sh k", k=W),
                in0=dk[:].rearrange("p (sh k) -> p sh k", k=W),
                in1=ub[:].unsqueeze(2).to_broadcast([128, NSUB * 10, W]),
                op=OP.is_le)
            wz = qpool.tile([128, NSUB * 10 * W], F32, tag="wz", name="wz")
            vec.tensor_tensor(out=wz[:], in0=hat[:], in1=vub[:], op=OP.mult)

            # mask assembly: m[s, hp, dx, dy, k]
            wc6 = wcorn[:].rearrange("p (s hp a d) -> p s hp a d",
                                     s=NSUB, hp=10, a=3)
            wx = wc6[:, :, :, 0, :]                     # (s, hp, dx)
            wy = wc6[:, :, :, 1, :]                     # (s, hp, dy)
            t1 = qpool.tile([128, NSUB * 40], F32, tag="t1", name="t1")
            vec.tensor_tensor(
                out=t1[:].rearrange("p (s hp x y) -> p s hp x y",
                                    s=NSUB, hp=10, x=2),
                in0=wx.unsqueeze(4).to_broadcast([128, NSUB, 10, 2, 2]),
                in1=wy.unsqueeze(3).to_broadcast([128, NSUB, 10, 2, 2]),
                op=OP.mult)
            t2 = qpool.tile([128, NSUB * 40], F32, tag="t2", name="t2")
            aw4 = aw[:].rearrange("p (s hp) -> p s hp", s=NSUB) \
                .unsqueeze(3).to_broadcast([128, NSUB, 10, 4])
            vec.tensor_tensor(
                out=t2[:].rearrange("p (s hp xy) -> p s hp xy", s=NSUB, hp=10),
                in0=t1[:].rearrange("p (s hp xy) -> p s hp xy", s=NSUB, hp=10),
                in1=aw4, op=OP.mult)
            mask = qpool.tile([128, NSUB * 40 * W], F32, tag="mask", name="mask")
            vec.tensor_tensor(
                out=mask[:].rearrange("p (shp xy k) -> p shp xy k", xy=4, k=W),
                in0=t2[:].rearrange("p (shp xy) -> p shp xy", xy=4)
                    .unsqueeze(3).to_broadcast([128, NSUB * 10, 4, W]),
                in1=wz[:].rearrange("p (shp k) -> p shp k", k=W)
                    .unsqueeze(2).to_broadcast([128, NSUB * 10, 4, W]),
                op=OP.mult)
            maskb = qpool.tile([128, NSUB * 40 * W], VDT, tag="maskb", name="maskb")
            vec.tensor_copy(out=maskb[:], in_=mask[:])
            if DEBUG and g == 0:
                nc.sync.dma_start(out=dbg["d_mask"][:], in_=mask[:])

            # gather row indices: 576*(gx-64) + 12*(gy-64) + rzq
            gx = gc6[:, :, :, 0, :]                     # (s, hp, dx)
            gy = gc6[:, :, :, 1, :]                     # (s, hp, dy)
            ga = qpool.tile([128, NSUB * 20], F32, tag="ga", name="ga")
            vec.tensor_scalar(out=ga[:].rearrange("p (s hp d) -> p s hp d",
                                                  s=NSUB, hp=10),
                              in0=gx, scalar1=576.0, scalar2=37632.0,
                              op0=OP.mult, op1=OP.subtract)
            gb = qpool.tile([128, NSUB * 20], F32, tag="gb", name="gb")
            vec.tensor_scalar(out=gb[:].rearrange("p (s hp d) -> p s hp d",
                                                  s=NSUB, hp=10),
                              in0=gy, scalar1=12.0, scalar2=None, op0=OP.mult)
            t4 = qpool.tile([128, NSUB * 40], F32, tag="t4", name="t4")
            vec.tensor_tensor(
                out=t4[:].rearrange("p (s hp x y) -> p s hp x y",
                                    s=NSUB, hp=10, x=2),
                in0=ga[:].rearrange("p (s hp x) -> p s hp x", s=NSUB, hp=10)
                    .unsqueeze(4).to_broadcast([128, NSUB, 10, 2, 2]),
                in1=gb[:].rearrange("p (s hp y) -> p s hp y", s=NSUB, hp=10)
                    .unsqueeze(3).to_broadcast([128, NSUB, 10, 2, 2]),
                op=OP.add)
            idxf = qpool.tile([128, NSUB * 40], F32, tag="idxf", name="idxf")
            vec.tensor_tensor(
                out=idxf[:].rearrange("p (s hp xy) -> p s hp xy", s=NSUB, hp=10),
                in0=t4[:].rearrange("p (s hp xy) -> p s hp xy", s=NSUB, hp=10),
                in1=rzq[:].rearrange("p (s hp) -> p s hp", s=NSUB)
                    .unsqueeze(3).to_broadcast([128, NSUB, 10, 4]),
                op=OP.add)
            idx16 = qpool.tile([128, NSUB * 40], I16, tag="idx16", name="idx16")
            vec.tensor_copy(out=idx16[:], in_=idxf[:])
            if DEBUG and g == 0:
                nc.sync.dma_start(out=dbg["d_idxf"][:], in_=idxf[:])

            # idx shuffle via DRAM: store [128, 160], reload 16-partition-wrapped
            nc.sync.dma_start(out=idxscr[g * 128:(g + 1) * 128, :], in_=idx16[:])
            idxw = gpool.tile([128, NSUB * 2 * 160], I16, tag="idxw", name="idxw")
            scr = idxscr[g * 128:(g + 1) * 128, :]
            for grp in range(8):
                # dest[p, sub, hl, r*8+s2] = scr[s2*16+p, sub*40+hl*20+r]
                src = bass.AP(scr.tensor, scr.offset,
                              [[160, 16], [40, NSUB], [20, 2], [1, 20],
                               [16 * 160, 8]])
                dst = idxw[grp * 16:(grp + 1) * 16, :] \
                    .rearrange("p (sub hl r s2) -> p sub hl r s2",
                               sub=NSUB, hl=2, r=20)
                nc.sync.dma_start(out=dst, in_=src)

            # gather + weighted reduce per (subtile, head)
            S = qpool.tile([128, NSUB * 64], F32, tag="S", name="S")
            for s in range(NSUB):
                for hl in range(2):
                    G = gpool.tile([128, 20 * W * 32], VDT, tag="G", name="G")
                    in_g = bass.AP(vflat[:].tensor, hl * BSV * 32,
                                   [[SROW * 32, NROWH], [1, W * 32]])
                    nc.gpsimd.dma_gather(
                        out_ap=G[:].rearrange("p (i e) -> p i e", i=20),
                        in_ap=in_g,
                        idxs_ap=idxw[:, (s * 2 + hl) * 160:(s * 2 + hl + 1) * 160],
                        num_idxs=NIDX, num_idxs_reg=NIDX,
                        elem_size=W * 32, elem_step=SROW * 32,
                        single_packet=False)
                    if DEBUG and g == 0 and s == 0 and hl == 0:
                        nc.sync.dma_start(out=dbg["d_G"][:], in_=G[:])
                    Pt = gpool.tile([128, 20 * W * 32], VDT, tag="Pt", name="Pt")
                    moff = s * (40 * W) + hl * (5 * 4 * W)
                    mg = bass.AP(maskb[:].tensor, maskb[:].offset + moff,
                                 [[NSUB * 40 * W, 128], [1, 20 * W], [0, 32]])
                    vec.tensor_tensor(
                        out=Pt[:].rearrange("p (m c) -> p m c", c=32),
                        in0=G[:].rearrange("p (m c) -> p m c", c=32),
                        in1=mg, op=OP.mult)
                    vec.tensor_reduce(
                        out=S[:, s * 64 + hl * 32:s * 64 + hl * 32 + 32],
                        in_=Pt[:].rearrange("p (m c) -> p c m", c=32),
                        axis=AX.X, op=OP.add)
            if DEBUG and g == 0:
                nc.sync.dma_start(out=dbg["d_S"][:], in_=S[:, 0:64])

            # transpose S [128, 64] -> [64, 128] per subtile
            for s in range(NSUB):
                pst = ps_t.tile([64, 128], F32, tag="pst", name="pst")
                nc.tensor.transpose(pst[:], S[:, s * 64:(s + 1) * 64], ident[:])
                act.activation(out=st_sb[:, q0 + s * 128:q0 + (s + 1) * 128],
                               in_=pst[:], func=ACT.Copy)

        # ---- GEMM2: outT = wout^T @ ST ----
        for mc in range(2):
            for ntile in range(NQ // 512):
                ps2 = ps_c.tile([128, 512], F32, tag="ps2", name="ps2")
                nc.tensor.matmul(ps2[:],
                                 wout_sb[:, mc * 128:(mc + 1) * 128],
                                 st_sb[:, ntile * 512:(ntile + 1) * 512],
                                 start=True, stop=True)
                ob = opool.tile([128, 512], F32, tag="ob", name="ob")
                vec.tensor_copy(out=ob[:], in_=ps2[:])
                nc.sync.dma_start(
                    out=outp[mc * 128:(mc + 1) * 128,
                             ntile * 512:(ntile + 1) * 512],
                    in_=ob[:])

    nc.compile()
    return nc


def _prep_core_inputs(inputs, b, j):
    q = np.ascontiguousarray(inputs["query"][b].T, np.float32)
    p = np.ascontiguousarray(inputs["pos"][b].T, np.float32)
    r = np.concatenate([inputs["reference_points"][b].T,
                        np.ones((1, NQ), np.float32)]).astype(np.float32)
    r = np.ascontiguousarray(r)
    value = np.ascontiguousarray(inputs["value"][b].reshape(C, NVOX), np.float32)

    W_off, b_off = inputs["W_off"], inputs["b_off"]
    W_attn, b_attn = inputs["W_attn"], inputs["b_attn"]
    heads = [2 * j, 2 * j + 1]
    rows, biases, refr = [], [], []
    for h in heads:
        for pp in range(P):
            for ax in range(3):
                rows.append(W_off[(h * P + pp) * 3 + ax])
                biases.append(b_off[(h * P + pp) * 3 + ax] - 0.5 + 64.0)
                e = np.zeros(3, np.float32)
                e[ax] = GRID
                refr.append(e)
    for h in heads:
        for pp in range(P):
            rows.append(W_attn[h * P + pp])
            biases.append(b_attn[h * P + pp])
            refr.append(np.zeros(3, np.float32))
    wcat = np.ascontiguousarray(np.stack(rows).T, np.float32)       # (256, 40)
    ref_rhs = np.concatenate(
        [np.stack(refr).T, np.asarray(biases, np.float32)[None, :]])
    ref_rhs = np.ascontiguousarray(ref_rhs, np.float32)             # (4, 40)

    wval = np.ascontiguousarray(inputs["W_val"][64 * j:64 * j + 64].T, np.float32)
    bval = np.ascontiguousarray(
        np.repeat(inputs["b_val"][64 * j:64 * j + 64][None, :], 128, axis=0),
        np.float32)
    wout = np.ascontiguousarray(inputs["W_out"][:, 64 * j:64 * j + 64].T,
                                np.float32)
    zoffs = np.repeat(np.arange(W, dtype=np.float32)[None, :], 128, axis=0)
    return {
        "value_in": value, "qT": q, "pT": p, "refT": r,
        "wcat": wcat, "ref_rhs": ref_rhs,
        "wval": wval, "bval": bval, "wout": wout, "zoff": zoffs,
    }


def get_nc():
    global _NC_CACHE
    if _NC_CACHE is None:
        _NC_CACHE = build_nc()
    return _NC_CACHE


def kernel(**inputs):
    from concourse.bass_utils import run_bass_kernel_spmd

    inputs = {k: np.asarray(v) for k, v in inputs.items()}
    nc = get_nc()
    in_maps = [_prep_core_inputs(inputs, core // 4, core % 4) for core in range(8)]
    res = run_bass_kernel_spmd(nc, in_maps, list(range(8)))
    bs = inputs["query"].shape[0]
    out = np.zeros((bs, NQ, C), np.float32)
    for core in range(8):
        out[core // 4] += res.results[core]["outp"].T
    out += inputs["b_out"][None, None, :].astype(np.float32)
    return out



# revision 21
# speedup vs baseline: 2.4840x; 2.4840x over previous
"""Trainium2 Bass kernel for 3D deformable attention (8 NeuronCores).

Sharding: core i handles batch b = i // 4 and head-pair j = i % 4
(heads 2j, 2j+1, i.e. value/out channels [64j, 64j+64)).

Per-core pipeline (v2 — DMA-packet-aware rewrite of the baseline):
  A. coords = qs^T @ Wcat^T + [48*ref | ones*bias] (PE) for all 8 query
     supertiles upfront; gather-row indices (DVE) and the 16-partition-
     wrapped index layout dma_gather needs, built ON-CHIP via 8 selector
     matmuls (fp32, exact for ints < 2^24) + one strided DVE copy.
     (The baseline bounced indices through DRAM with a 2-byte-granular
     AP — 1.3M two-byte DMA packets that choked all 16 DMA engines.)
  B. value projection in [128, 8192]-voxel slabs: fp32 loads with 32KB
     per-partition packets split across the SP and ACT HWDGE queues,
     ACT casts to bf16, voxel-stationary bf16 matmuls arranged so psum
     partition j owns voxels [64j, 64j+64) -> vflat writes are 4KB runs
     issued on the Pool (SWDGE) queue.  Softmax/corner/mask weights
     (DVE) overlap the slab DMA.
  C. per (query-subtile, head): dma_gather of 2560 rows (8 vox x 32ch
     bf16 each), then a contiguous bf16 fold tree (x wz, fold k 8->1,
     x aw*wx*wy, fp32 reduce over 20 points/corners) instead of the
     baseline's 64B-strided fp32 reduce.
  D. PE transpose of S, outT = Wout_cols^T @ S^T, DMA out.
Host combines: out[b] = sum_j outp_j^T + b_out.
"""
import numpy as np

import concourse.bass as bass
import concourse.mybir as mybir
from concourse import bacc, tile
from concourse.masks import make_identity
from contextlib import ExitStack

F32 = mybir.dt.float32
BF16 = mybir.dt.bfloat16
I16 = mybir.dt.int16
AX = mybir.AxisListType
OP = mybir.AluOpType
ACT = mybir.ActivationFunctionType

H, P = 8, 5
NQ, C, GRID = 4096, 256, 48
NVOX = GRID ** 3            # 110592
NSUB = 4                    # query subtiles (of 128) per supertile
TQ = 128 * NSUB             # 512
NSUP = NQ // TQ             # 8

VDT = BF16
W = 8                       # voxels per gathered row
SROW = 4                    # voxels per row step (4*32*2B == 256B)
NROWH = NVOX // SROW        # 27648 rows per head (< 32768 for int16)
BSV = NVOX + 8              # voxels per head block incl pad
NIDX = 20 * 128             # rows per (subtile, head) gather

SLAB = 8192                 # voxels per value-proj slab
NSLAB = 14                  # 13 full + 1 half (4096)

DEBUG = False
USE_INDIRECT = False        # gather via gpsimd indirect_dma_start (1 instr)

_NC_CACHE = None


def build_nc():
    nc = bacc.Bacc("TRN2", target_bir_lowering=False, debug=False, num_devices=8)

    value_in = nc.dram_tensor("value_in", [C, NVOX], F32, kind="ExternalInput")
    qT = nc.dram_tensor("qT", [C, NQ], F32, kind="ExternalInput")
    pT = nc.dram_tensor("pT", [C, NQ], F32, kind="ExternalInput")
    refT = nc.dram_tensor("refT", [4, NQ], F32, kind="ExternalInput")
    wcat = nc.dram_tensor("wcat", [C, 40], F32, kind="ExternalInput")
    ref_rhs = nc.dram_tensor("ref_rhs", [4, 40], F32, kind="ExternalInput")
    wvalb = nc.dram_tensor("wvalb", [C, 64], BF16, kind="ExternalInput")
    bval = nc.dram_tensor("bval", [128, 64], F32, kind="ExternalInput")
    wout = nc.dram_tensor("wout", [64, C], F32, kind="ExternalInput")
    zoff = nc.dram_tensor("zoff", [128, W], F32, kind="ExternalInput")
    selmats = nc.dram_tensor("selmats", [128, 8 * 128], F32, kind="ExternalInput")
    outp = nc.dram_tensor("outp", [C, NQ], F32, kind="ExternalOutput")
    vflat = nc.dram_tensor("vflat", [2 * BSV * 32], VDT)

    dbg = {}
    if DEBUG:
        dbg["d_idxw"] = nc.dram_tensor("d_idxw", [128, 1280], I16,
                                       kind="ExternalOutput")
        dbg["d_mask"] = nc.dram_tensor("d_mask", [128, 1280], VDT,
                                       kind="ExternalOutput")
        dbg["d_S"] = nc.dram_tensor("d_S", [128, 256], F32,
                                    kind="ExternalOutput")
        dbg["d_vfs"] = nc.dram_tensor("d_vfs", [8192 * 32], VDT,
                                      kind="ExternalOutput")

    vec = nc.vector
    act = nc.scalar

    with tile.TileContext(nc) as tc, ExitStack() as ctx:
        const = ctx.enter_context(tc.tile_pool(name="const", bufs=1))
        keep = ctx.enter_context(tc.tile_pool(name="keep", bufs=1))

        # ---- constants into SBUF ----
        wcat_sb = [const.tile([128, 40], F32, tag=f"wcat{k}", name=f"wcat{k}")
                   for k in range(2)]
        for k in range(2):
            nc.sync.dma_start(out=wcat_sb[k][:], in_=wcat[k * 128:(k + 1) * 128, :])
        refrhs_sb = const.tile([4, 40], F32, tag="refrhs", name="refrhs")
        nc.sync.dma_start(out=refrhs_sb[:], in_=ref_rhs[:])
        wval_sb = [const.tile([128, 64], BF16, tag=f"wval{k}", name=f"wval{k}")
                   for k in range(2)]
        for k in range(2):
            nc.sync.dma_start(out=wval_sb[k][:], in_=wvalb[k * 128:(k + 1) * 128, :])
        bval_sb = const.tile([128, 64], F32, tag="bval", name="bval")
        nc.sync.dma_start(out=bval_sb[:], in_=bval[:])
        wout_sb = const.tile([64, C], F32, tag="wout", name="wout")
        nc.sync.dma_start(out=wout_sb[:], in_=wout[:])
        zoff_sb = const.tile([128, W], F32, tag="zoff", name="zoff")
        nc.sync.dma_start(out=zoff_sb[:], in_=zoff[:])
        sel_sb = const.tile([128, 8 * 128], F32, tag="sel", name="sel")
        nc.scalar.dma_start(out=sel_sb[:], in_=selmats[:])
        ident = const.tile([128, 128], F32, tag="ident", name="ident")
        make_identity(nc, ident[:])

        # persistent across phases
        coords_all = keep.tile([128, NSUP * 160], F32, tag="coords", name="coords")
        if USE_INDIRECT:
            idxn_all = keep.tile([128, NSUP * 160], I16, tag="idxn", name="idxn")
        else:
            idxw_all = keep.tile([128, NSUP * 1280], I16, tag="idxw",
                                 name="idxw")
        rzq_all = keep.tile([128, NSUP * 40], F32, tag="rzq", name="rzq")
        mask_all = keep.tile([128, NSUP * 1280], BF16, tag="mask", name="mask")
        st_sb = keep.tile([64, NQ], F32, tag="st", name="st")

        # ================= stage A: coords + wrapped gather indices ========
        with tc.tile_pool(name="qpool", bufs=2) as qpool, \
             tc.tile_pool(name="apool", bufs=2) as apool, \
             tc.tile_pool(name="ps_c", bufs=2, space="PSUM") as ps_c, \
             tc.tile_pool(name="ps_w", bufs=1, space="PSUM") as ps_w:

            qs_sb = [qpool.tile([128, NQ], F32, tag=f"qs{k}", name=f"qs{k}")
                     for k in range(2)]
            ref_sb = qpool.tile([4, NQ], F32, tag="refq", name="refq")
            for k in range(2):
                for half in range(2):
                    sl = slice(half * (NQ // 2), (half + 1) * (NQ // 2))
                    ptmp = apool.tile([128, NQ // 2], F32, tag="ptmp", name="ptmp")
                    nc.sync.dma_start(out=qs_sb[k][:, sl],
                                      in_=qT[k * 128:(k + 1) * 128, sl])
                    nc.scalar.dma_start(out=ptmp[:], in_=pT[k * 128:(k + 1) * 128, sl])
                    vec.tensor_tensor(out=qs_sb[k][:, sl], in0=qs_sb[k][:, sl],
                                      in1=ptmp[:], op=OP.add)
            nc.sync.dma_start(out=ref_sb[:], in_=refT[:])

            for g in range(NSUP):
                q0 = g * TQ
                psc = ps_c.tile([128, 160], F32, tag="psc", name="psc")
                for s in range(NSUB):
                    qsl = slice(q0 + s * 128, q0 + (s + 1) * 128)
                    nc.tensor.matmul(psc[:, s * 40:(s + 1) * 40],
                                     qs_sb[0][:, qsl], wcat_sb[0][:],
                                     start=True, stop=False)
                    nc.tensor.matmul(psc[:, s * 40:(s + 1) * 40],
                                     qs_sb[1][:, qsl], wcat_sb[1][:],
                                     start=False, stop=False)
                    nc.tensor.matmul(psc[:, s * 40:(s + 1) * 40],
                                     ref_sb[:, qsl], refrhs_sb[:],
                                     start=False, stop=True)
                co_g = coords_all[:, g * 160:(g + 1) * 160]
                act.activation(out=co_g, in_=psc[:], func=ACT.Copy)

                # --- index math (DVE) ---
                co = co_g.rearrange("p (s r) -> p s r", s=NSUB)
                pix = co[:, :, 0:30]
                NPX = NSUB * 30
                flo = apool.tile([128, NPX], F32, tag="flo", name="flo")
                vec.tensor_scalar(out=flo[:].rearrange("p (s r) -> p s r", s=NSUB),
                                  in0=pix, scalar1=8388607.5, scalar2=8388608.0,
                                  op0=OP.add, op1=OP.subtract)
                fl3 = flo[:].rearrange("p (s r) -> p s r", s=NSUB)
                gcorn = apool.tile([128, NPX * 2], F32, tag="gcorn", name="gcorn")
                gc4 = gcorn[:].rearrange("p (s r d) -> p s r d", s=NSUB, d=2)
                vec.tensor_scalar(out=gc4[:, :, :, 0], in0=fl3,
                                  scalar1=64.0, scalar2=111.0,
                                  op0=OP.max, op1=OP.min)
                g1m = apool.tile([128, NPX], F32, tag="g1m", name="g1m")
                vec.tensor_scalar(out=g1m[:], in0=flo[:],
                                  scalar1=63.0, scalar2=110.0,
                                  op0=OP.max, op1=OP.min)
                vec.tensor_scalar(out=gc4[:, :, :, 1], in0=g1m[:]
                                  .rearrange("p (s r) -> p s r", s=NSUB),
                                  scalar1=1.0, scalar2=None, op0=OP.add)
                gc6 = gcorn[:].rearrange("p (s hp a d) -> p s hp a d",
                                         s=NSUB, hp=10, a=3)
                gz = gc6[:, :, :, 2, 0]
                rzq_g = rzq_all[:, g * 40:(g + 1) * 40]
                tq = apool.tile([128, NSUB * 10], F32, tag="tq", name="tq")
                vec.tensor_scalar(out=tq[:].rearrange("p (s h) -> p s h", s=NSUB),
                                  in0=gz, scalar1=0.25, scalar2=16.375,
                                  op0=OP.mult, op1=OP.subtract)
                vec.tensor_scalar(out=rzq_g, in0=tq[:],
                                  scalar1=8388624.0, scalar2=8388624.0,
                                  op0=OP.add, op1=OP.subtract)
                gx = gc6[:, :, :, 0, :]
                gy = gc6[:, :, :, 1, :]
                # Components for the gather row index, kept small (<= 112)
                # so the PE shuffle matmuls below are exact regardless of
                # the PE's internal fp32 mantissa width.
                gxe = apool.tile([128, NSUB * 40], F32, tag="gxe", name="gxe")
                vec.tensor_copy(
                    out=gxe[:].rearrange("p (s hp x y) -> p s hp x y",
                                         s=NSUB, hp=10, x=2),
                    in_=gx.unsqueeze(4).to_broadcast([128, NSUB, 10, 2, 2]))
                gye = apool.tile([128, NSUB * 40], F32, tag="gye", name="gye")
                vec.tensor_copy(
                    out=gye[:].rearrange("p (s hp x y) -> p s hp x y",
                                         s=NSUB, hp=10, x=2),
                    in_=gy.unsqueeze(3).to_broadcast([128, NSUB, 10, 2, 2]))
                rze = apool.tile([128, NSUB * 40], F32, tag="rze", name="rze")
                vec.tensor_copy(
                    out=rze[:].rearrange("p (s hp xy) -> p s hp xy",
                                         s=NSUB, hp=10),
                    in_=rzq_g.rearrange("p (s hp) -> p s hp", s=NSUB)
                        .unsqueeze(3).to_broadcast([128, NSUB, 10, 4]))

                if USE_INDIRECT:
                    # natural-layout row indices; indirect DMA needs no wrap
                    idf = apool.tile([128, 160], F32, tag="idf", name="idf")
                    vec.tensor_scalar(out=idf[:], in0=gxe[:],
                                      scalar1=576.0, scalar2=37632.0,
                                      op0=OP.mult, op1=OP.subtract)
                    vec.scalar_tensor_tensor(out=idf[:], in0=gye[:],
                                             scalar=12.0, in1=idf[:],
                                             op0=OP.mult, op1=OP.add)
                    vec.tensor_tensor(out=idf[:], in0=rze[:], in1=idf[:],
                                      op=OP.add)
                    vec.tensor_copy(out=idxn_all[:, g * 160:(g + 1) * 160],
                                    in_=idf[:])
                    continue
                # --- wrapped idx: shuffle each component via 8 selector
                # matmuls, then assemble 576*gx + 12*gy + rzq - 37632 on DVE.
                # psum blocks padded to 256-col stride: a matmul output
                # region must not cross a 2KB psum bank boundary.
                wf = apool.tile([128, 1280], F32, tag="wf", name="wf")
                for ci, comp in enumerate((gxe, gye, rze)):
                    psw = ps_w.tile([128, 8 * 256], F32, tag="psw", name="psw")
                    for s2 in range(8):
                        nc.tensor.matmul(psw[:, s2 * 256:s2 * 256 + 160],
                                         sel_sb[:, s2 * 128:(s2 + 1) * 128],
                                         comp[:], start=True, stop=True)
                    pswv = psw[:].rearrange("p (s2 x) -> p s2 x", s2=8)[:, :, 0:160]
                    wfv = wf[:].rearrange("p (s2 x) -> p s2 x", s2=8)
                    if ci == 0:
                        vec.tensor_scalar(out=wfv, in0=pswv,
                                          scalar1=576.0, scalar2=37632.0,
                                          op0=OP.mult, op1=OP.subtract)
                    elif ci == 1:
                        vec.scalar_tensor_tensor(out=wfv, in0=pswv,
                                                 scalar=12.0, in1=wfv,
                                                 op0=OP.mult, op1=OP.add)
                    else:
                        vec.tensor_tensor(out=wfv, in0=pswv, in1=wfv,
                                          op=OP.add)
                # idxw[p, (s,hl,r,s2)] = wf[p, (s2, s, hl, r)]  (cast to i16)
                idxw_g = idxw_all[:, g * 1280:(g + 1) * 1280] \
                    .rearrange("p (s hl r s2) -> p s hl r s2",
                               s=NSUB, hl=2, r=20)
                for s2 in range(8):
                    vec.tensor_copy(
                        out=idxw_g[:, :, :, :, s2],
                        in_=wf[:, s2 * 160:(s2 + 1) * 160]
                            .rearrange("p (s hl r) -> p s hl r", s=NSUB, hl=2))
                if DEBUG and g == 0:
                    nc.sync.dma_start(out=dbg["d_idxw"][:],
                                      in_=idxw_all[:, 0:1280])

        # ================= stage B: value projection =======================
        # (weight/mask math for stage C interleaved below to overlap DMA)
        with tc.tile_pool(name="vpool", bufs=2) as vpool, \
             tc.tile_pool(name="bpool", bufs=1) as bpool, \
             tc.tile_pool(name="spool", bufs=2) as spool, \
             tc.tile_pool(name="mpool", bufs=2) as mpool, \
             tc.tile_pool(name="ps_v", bufs=1, space="PSUM") as ps_v:

            zpad = spool.tile([8, 32], VDT, tag="zpad", name="zpad")
            vec.memset(zpad[:], 0.0)
            vflat_r = vflat[:].rearrange("(v c) -> v c", c=32)
            for hl in range(2):
                nc.gpsimd.dma_start(
                    out=vflat_r[hl * BSV + NVOX:hl * BSV + NVOX + 8, :],
                    in_=zpad[:])

            def stage_c(g):
                """softmax + corner weights + z-hat for supertile g (DVE/ACT)."""
                co = coords_all[:, g * 160:(g + 1) * 160] \
                    .rearrange("p (s r) -> p s r", s=NSUB)
                pix = co[:, :, 0:30]
                logit = co[:, :, 30:40]
                exlog = mpool.tile([128, NSUB * 10], F32, tag="exlog", name="exlog")
                act.activation(out=exlog[:], in_=logit, func=ACT.Exp)
                ex4 = exlog[:].rearrange("p (s h q) -> p s h q", s=NSUB, h=2)
                sums = mpool.tile([128, NSUB * 2], F32, tag="sums", name="sums")
                vec.tensor_reduce(out=sums[:].rearrange("p (s h) -> p s h", s=NSUB),
                                  in_=ex4, axis=AX.X, op=OP.add)
                rsum = mpool.tile([128, NSUB * 2], F32, tag="rsum", name="rsum")
                vec.reciprocal(out=rsum[:], in_=sums[:])
                aw = mpool.tile([128, NSUB * 10], F32, tag="aw", name="aw")
                vec.tensor_tensor(
                    out=aw[:].rearrange("p (sh q) -> p sh q", q=5),
                    in0=exlog[:].rearrange("p (sh q) -> p sh q", q=5),
                    in1=rsum[:].unsqueeze(2).to_broadcast([128, NSUB * 2, 5]),
                    op=OP.mult)

                NPX = NSUB * 30
                flo = mpool.tile([128, NPX], F32, tag="cflo", name="cflo")
                vec.tensor_scalar(out=flo[:].rearrange("p (s r) -> p s r", s=NSUB),
                                  in0=pix, scalar1=8388607.5, scalar2=8388608.0,
                                  op0=OP.add, op1=OP.subtract)
                fl3 = flo[:].rearrange("p (s r) -> p s r", s=NSUB)
                frac = mpool.tile([128, NPX], F32, tag="cfrac", name="cfrac")
                vec.tensor_tensor(out=frac[:].rearrange("p (s r) -> p s r", s=NSUB),
                                  in0=pix, in1=fl3, op=OP.subtract)
                fr3 = frac[:].rearrange("p (s r) -> p s r", s=NSUB)
                gcl = mpool.tile([128, NPX], F32, tag="cgcl", name="cgcl")
                vec.tensor_scalar(out=gcl[:], in0=flo[:],
                                  scalar1=64.0, scalar2=111.0,
                                  op0=OP.max, op1=OP.min)
                g1m = mpool.tile([128, NPX], F32, tag="cg1m", name="cg1m")
                vec.tensor_scalar(out=g1m[:], in0=flo[:],
                                  scalar1=63.0, scalar2=110.0,
                                  op0=OP.max, op1=OP.min)
                v0 = mpool.tile([128, NPX], F32, tag="cv0", name="cv0")
                vec.tensor_tensor(out=v0[:], in0=gcl[:], in1=flo[:],
                                  op=OP.is_equal)
                v1 = mpool.tile([128, NPX], F32, tag="cv1", name="cv1")
                vec.tensor_tensor(out=v1[:], in0=g1m[:], in1=flo[:],
                                  op=OP.is_equal)
                om = mpool.tile([128, NPX], F32, tag="com", name="com")
                vec.tensor_scalar(out=om[:], in0=frac[:], scalar1=-1.0,
                                  scalar2=1.0, op0=OP.mult, op1=OP.add)
                wcorn = mpool.tile([128, NPX * 2], F32, tag="cwc", name="cwc")
                wc4 = wcorn[:].rearrange("p (s r d) -> p s r d", s=NSUB, d=2)
                vec.tensor_tensor(out=wc4[:, :, :, 0],
                                  in0=om[:].rearrange("p (s r) -> p s r", s=NSUB),
                                  in1=v0[:].rearrange("p (s r) -> p s r", s=NSUB),
                                  op=OP.mult)
                vec.tensor_tensor(out=wc4[:, :, :, 1],
                                  in0=fr3,
                                  in1=v1[:].rearrange("p (s r) -> p s r", s=NSUB),
                                  op=OP.mult)

                # z-hat over W slots
                pz = pix.rearrange("p s (hp a) -> p s hp a", a=3)[:, :, :, 2]
                rzq_g = rzq_all[:, g * 40:(g + 1) * 40]
                zb4 = mpool.tile([128, NSUB * 10], F32, tag="czb4", name="czb4")
                vec.tensor_scalar(out=zb4[:], in0=rzq_g, scalar1=4.0,
                                  scalar2=64.0, op0=OP.mult, op1=OP.add)
                d0 = mpool.tile([128, NSUB * 10], F32, tag="cd0", name="cd0")
                vec.tensor_tensor(out=d0[:].rearrange("p (s h) -> p s h", s=NSUB),
                                  in0=zb4[:].rearrange("p (s h) -> p s h", s=NSUB),
                                  in1=pz, op=OP.subtract)
                dk = mpool.tile([128, NSUB * 10 * W], F32, tag="cdk", name="cdk")
                vec.tensor_tensor(
                    out=dk[:].rearrange("p (sh k) -> p sh k", k=W),
                    in0=d0[:].unsqueeze(2).to_broadcast([128, NSUB * 10, W]),
                    in1=zoff_sb[:].unsqueeze(1).to_broadcast([128, NSUB * 10, W]),
                    op=OP.add)
                adk = mpool.tile([128, NSUB * 10 * W], F32, tag="cadk", name="cadk")
                act.activation(out=adk[:], in_=dk[:], func=ACT.Abs)
                hat = mpool.tile([128, NSUB * 10 * W], F32, tag="chat", name="chat")
                act.activation(out=hat[:], in_=adk[:], func=ACT.Relu,
                               scale=-1.0, bias=1.0)
                ub = mpool.tile([128, NSUB * 10], F32, tag="cub", name="cub")
                vec.tensor_scalar(out=ub[:].rearrange("p (s h) -> p s h", s=NSUB),
                                  in0=pz, scalar1=-1.0, scalar2=111.0,
                                  op0=OP.mult, op1=OP.add)
                vub = mpool.tile([128, NSUB * 10 * W], F32, tag="cvub", name="cvub")
                vec.tensor_tensor(
                    out=vub[:].rearrange("p (sh k) -> p sh k", k=W),
                    in0=dk[:].rearrange("p (sh k) -> p sh k", k=W),
                    in1=ub[:].unsqueeze(2).to_broadcast([128, NSUB * 10, W]),
                    op=OP.is_le)
                wz = mpool.tile([128, NSUB * 10 * W], F32, tag="cwz", name="cwz")
                vec.tensor_tensor(out=wz[:], in0=hat[:], in1=vub[:], op=OP.mult)

                # mask = aw * wx * wy * wz  (bf16), layout (s, hp, xy, k)
                wc6 = wcorn[:].rearrange("p (s hp a d) -> p s hp a d",
                                         s=NSUB, hp=10, a=3)
                wx = wc6[:, :, :, 0, :]
                wy = wc6[:, :, :, 1, :]
                t1 = mpool.tile([128, NSUB * 40], F32, tag="ct1", name="ct1")
                vec.tensor_tensor(
                    out=t1[:].rearrange("p (s hp x y) -> p s hp x y",
                                        s=NSUB, hp=10, x=2),
                    in0=wx.unsqueeze(4).to_broadcast([128, NSUB, 10, 2, 2]),
                    in1=wy.unsqueeze(3).to_broadcast([128, NSUB, 10, 2, 2]),
                    op=OP.mult)
                t2 = mpool.tile([128, NSUB * 40], F32, tag="ct2", name="ct2")
                aw4 = aw[:].rearrange("p (s hp) -> p s hp", s=NSUB) \
                    .unsqueeze(3).to_broadcast([128, NSUB, 10, 4])
                vec.tensor_tensor(
                    out=t2[:].rearrange("p (s hp xy) -> p s hp xy",
                                        s=NSUB, hp=10),
                    in0=t1[:].rearrange("p (s hp xy) -> p s hp xy",
                                        s=NSUB, hp=10),
                    in1=aw4, op=OP.mult)
                mask_g = mask_all[:, g * 1280:(g + 1) * 1280]
                vec.tensor_tensor(
                    out=mask_g.rearrange("p (shp xy k) -> p shp xy k",
                                         xy=4, k=W),
                    in0=t2[:].rearrange("p (shp xy) -> p shp xy", xy=4)
                        .unsqueeze(3).to_broadcast([128, NSUB * 10, 4, W]),
                    in1=wz[:].rearrange("p (shp k) -> p shp k", k=W)
                        .unsqueeze(2).to_broadcast([128, NSUB * 10, 4, W]),
                    op=OP.mult)
                if DEBUG and g == 0:
                    nc.sync.dma_start(out=dbg["d_mask"][:],
                                      in_=mask_all[:, 0:1280])

            CH = 4096                   # fp32 load chunk (16KB/partition)
            for slab in range(NSLAB):
                if slab < 8:
                    stage_c(slab)
                off = slab * SLAB
                size = min(SLAB, NVOX - off)
                R = size // 128
                vb = [bpool.tile([128, SLAB], BF16, tag=f"vb{k}", name=f"vb{k}")
                      for k in range(2)]
                for c0 in range(0, size, CH):
                    for k in range(2):
                        vin = vpool.tile([128, CH], F32, tag=f"vin{k}",
                                         name=f"vin{k}")
                        eng = nc.sync if k == 0 else nc.scalar
                        eng.dma_start(
                            out=vin[:],
                            in_=value_in[k * 128:(k + 1) * 128,
                                         off + c0:off + c0 + CH])
                        act.activation(out=vb[k][:, c0:c0 + CH], in_=vin[:],
                                       func=ACT.Copy)
                psv = ps_v.tile([128, 4096], F32, tag="psv", name="psv")
                for t in range(R):
                    lhs0 = vb[0][:, 0:size].rearrange("p (j r) -> p r j", r=R)[:, t, :]
                    lhs1 = vb[1][:, 0:size].rearrange("p (j r) -> p r j", r=R)[:, t, :]
                    nc.tensor.matmul(psv[:, t * 64:(t + 1) * 64], lhs0,
                                     wval_sb[0][:], start=True, stop=False)
                    nc.tensor.matmul(psv[:, t * 64:(t + 1) * 64], lhs1,
                                     wval_sb[1][:], start=False, stop=True)
                # drain per head: stg[j, (t,c)] = psv[j, t*64+hl*32+c] + bval
                psr = psv[:, 0:R * 64].rearrange("p (t hc) -> p t hc", t=R)
                for hl in range(2):
                    stg = spool.tile([128, (SLAB // 128) * 32], VDT,
                                     tag=f"stg{hl}", name=f"stg{hl}")
                    bv = bval_sb[:, hl * 32:(hl + 1) * 32] \
                        .unsqueeze(1).to_broadcast([128, R, 32])
                    vec.tensor_tensor(
                        out=stg[:, 0:R * 32].rearrange("p (t c) -> p t c", t=R),
                        in0=psr[:, :, hl * 32:(hl + 1) * 32],
                        in1=bv, op=OP.add)
                    dst = vflat_r[hl * BSV + off:hl * BSV + off + size, :] \
                        .rearrange("(j t) c -> j (t c)", j=128)
                    nc.gpsimd.dma_start(out=dst, in_=stg[:, 0:R * 32])

        if DEBUG:
            nc.sync.dma_start(out=dbg["d_vfs"][:], in_=vflat[0:8192 * 32])

        # ================= stage C/D: gather + weighted reduce =============
        with tc.tile_pool(name="gpool", bufs=3) as gpool, \
             tc.tile_pool(name="fpool", bufs=2) as fpool, \
             tc.tile_pool(name="opool", bufs=2) as opool, \
             tc.tile_pool(name="ps_t", bufs=2, space="PSUM") as ps_t, \
             tc.tile_pool(name="ps_o", bufs=2, space="PSUM") as ps_o:

            for g in range(NSUP):
                q0 = g * TQ
                S = fpool.tile([128, NSUB * 64], F32, tag="S", name="S")
                for s in range(NSUB):
                    for hl in range(2):
                        G = gpool.tile([128, 20 * W * 32], VDT, tag="G", name="G")
                        if USE_INDIRECT:
                            nrows = (2 * BSV * 32) // 128
                            src = bass.AP(vflat[:].tensor, 0,
                                          [[128, nrows], [1, 128]])
                            nc.gpsimd.indirect_dma_start(
                                out=G[:].rearrange("p (i e) -> p i e", i=20),
                                out_offset=None,
                                in_=src,
                                in_offset=bass.IndirectOffsetOnAxis(
                                    ap=idxn_all[:, g * 160 + (s * 2 + hl) * 20:
                                                g * 160 + (s * 2 + hl + 1) * 20],
                                    axis=0),
                                element_offset=hl * BSV * 32)
                        else:
                            in_g = bass.AP(vflat[:].tensor, hl * BSV * 32,
                                           [[SROW * 32, NROWH], [1, W * 32]])
                            nc.gpsimd.dma_gather(
                                out_ap=G[:].rearrange("p (i e) -> p i e", i=20),
                                in_ap=in_g,
                                idxs_ap=idxw_all[:, g * 1280 + (s * 2 + hl) * 160:
                                                 g * 1280 + (s * 2 + hl + 1) * 160],
                                num_idxs=NIDX, num_idxs_reg=NIDX,
                                elem_size=W * 32, elem_step=SROW * 32,
                                single_packet=False)
                        # Pt = G * mask  (bf16, contiguous)
                        Pt = gpool.tile([128, 20 * W * 32], VDT, tag="Pt",
                                        name="Pt")
                        moff = g * 1280 + s * 320 + hl * 160
                        wzap = bass.AP(mask_all[:].tensor,
                                       mask_all[:].offset + moff,
                                       [[NSUP * 1280, 128], [1, 160], [0, 32]])
                        vec.tensor_tensor(
                            out=Pt[:].rearrange("p (m c) -> p m c", c=32),
                            in0=G[:].rearrange("p (m c) -> p m c", c=32),
                            in1=wzap, op=OP.mult)
                        # fold k: 8 -> 4 -> 2 -> 1  (contiguous adds)
                        F1 = fpool.tile([128, 2560], VDT, tag="F1", name="F1")
                        vec.tensor_tensor(
                            out=F1[:].rearrange("p (i x) -> p i x", i=20),
                            in0=Pt[:].rearrange("p (i k x) -> p i k x",
                                                i=20, k=2)[:, :, 0, :],
                            in1=Pt[:].rearrange("p (i k x) -> p i k x",
                                                i=20, k=2)[:, :, 1, :],
                            op=OP.add)
                        F2 = fpool.tile([128, 1280], VDT, tag="F2", name="F2")
                        vec.tensor_tensor(
                            out=F2[:].rearrange("p (i x) -> p i x", i=20),
                            in0=F1[:].rearrange("p (i k x) -> p i k x",
                                                i=20, k=2)[:, :, 0, :],
                            in1=F1[:].rearrange("p (i k x) -> p i k x",
                                                i=20, k=2)[:, :, 1, :],
                            op=OP.add)
                        F3 = fpool.tile([128, 640], VDT, tag="F3", name="F3")
                        vec.tensor_tensor(
                            out=F3[:].rearrange("p (i x) -> p i x", i=20),
                            in0=F2[:].rearrange("p (i k x) -> p i k x",
                                                i=20, k=2)[:, :, 0, :],
                            in1=F2[:].rearrange("p (i k x) -> p i k x",
                                                i=20, k=2)[:, :, 1, :],
                            op=OP.add)
                        # reduce over the 20 (point, corner) rows (fp32 out)
                        vec.tensor_reduce(
                            out=S[:, s * 64 + hl * 32:s * 64 + hl * 32 + 32],
                            in_=F3[:].rearrange("p (i c) -> p c i", i=20),
                            axis=AX.X, op=OP.add)

                if DEBUG and g == 0:
                    nc.sync.dma_start(out=dbg["d_S"][:], in_=S[:])
                # transpose S [128, 64] -> [64, 128] per subtile
                for s in range(NSUB):
                    pst = ps_t.tile([64, 128], F32, tag="pst", name="pst")
                    nc.tensor.transpose(pst[:], S[:, s * 64:(s + 1) * 64],
                                        ident[:])
                    act.activation(out=st_sb[:, q0 + s * 128:q0 + (s + 1) * 128],
                                   in_=pst[:], func=ACT.Copy)

            # ---- GEMM2: outT = wout^T @ ST ----
            for mc in range(2):
                for ntile in range(NQ // 512):
                    ps2 = ps_o.tile([128, 512], F32, tag="ps2", name="ps2")
                    nc.tensor.matmul(ps2[:],
                                     wout_sb[:, mc * 128:(mc + 1) * 128],
                                     st_sb[:, ntile * 512:(ntile + 1) * 512],
                                     start=True, stop=True)
                    ob = opool.tile([128, 512], F32, tag="ob", name="ob")
                    vec.tensor_copy(out=ob[:], in_=ps2[:])
                    eng = nc.sync if ntile % 2 == 0 else nc.scalar
                    eng.dma_start(
                        out=outp[mc * 128:(mc + 1) * 128,
                                 ntile * 512:(ntile + 1) * 512],
                        in_=ob[:])

    nc.compile()
    return nc


def _prep_core_inputs(inputs, b, j):
    import ml_dtypes
    q = np.ascontiguousarray(inputs["query"][b].T, np.float32)
    p = np.ascontiguousarray(inputs["pos"][b].T, np.float32)
    r = np.concatenate([inputs["reference_points"][b].T,
                        np.ones((1, NQ), np.float32)]).astype(np.float32)
    r = np.ascontiguousarray(r)
    value = np.ascontiguousarray(inputs["value"][b].reshape(C, NVOX), np.float32)

    W_off, b_off = inputs["W_off"], inputs["b_off"]
    W_attn, b_attn = inputs["W_attn"], inputs["b_attn"]
    heads = [2 * j, 2 * j + 1]
    rows, biases, refr = [], [], []
    for h in heads:
        for pp in range(P):
            for ax in range(3):
                rows.append(W_off[(h * P + pp) * 3 + ax])
                biases.append(b_off[(h * P + pp) * 3 + ax] - 0.5 + 64.0)
                e = np.zeros(3, np.float32)
                e[ax] = GRID
                refr.append(e)
    for h in heads:
        for pp in range(P):
            rows.append(W_attn[h * P + pp])
            biases.append(b_attn[h * P + pp])
            refr.append(np.zeros(3, np.float32))
    wcat = np.ascontiguousarray(np.stack(rows).T, np.float32)       # (256, 40)
    ref_rhs = np.concatenate(
        [np.stack(refr).T, np.asarray(biases, np.float32)[None, :]])
    ref_rhs = np.ascontiguousarray(ref_rhs, np.float32)             # (4, 40)

    wvalb = np.ascontiguousarray(inputs["W_val"][64 * j:64 * j + 64].T) \
        .astype(ml_dtypes.bfloat16)
    bval = np.ascontiguousarray(
        np.repeat(inputs["b_val"][64 * j:64 * j + 64][None, :], 128, axis=0),
        np.float32)
    wout = np.ascontiguousarray(inputs["W_out"][:, 64 * j:64 * j + 64].T,
                                np.float32)
    zoffs = np.repeat(np.arange(W, dtype=np.float32)[None, :], 128, axis=0)
    # selector matrices: sel[q, s2*128 + p'] = 1 iff q == 16*s2 + (p' % 16)
    sel = np.zeros((128, 8, 128), np.float32)
    for s2 in range(8):
        for pp in range(128):
            sel[16 * s2 + (pp % 16), s2, pp] = 1.0
    sel = np.ascontiguousarray(sel.reshape(128, 8 * 128))
    return {
        "value_in": value, "qT": q, "pT": p, "refT": r,
        "wcat": wcat, "ref_rhs": ref_rhs,
        "wvalb": wvalb, "bval": bval, "wout": wout, "zoff": zoffs,
        "selmats": sel,
    }


def get_nc():
    global _NC_CACHE
    if _NC_CACHE is None:
        _NC_CACHE = build_nc()
    return _NC_CACHE


def kernel(**inputs):
    from concourse.bass_utils import run_bass_kernel_spmd

    inputs = {k: np.asarray(v) for k, v in inputs.items()}
    nc = get_nc()
    in_maps = [_prep_core_inputs(inputs, core // 4, core % 4) for core in range(8)]
    res = run_bass_kernel_spmd(nc, in_maps, list(range(8)))
    bs = inputs["query"].shape[0]
    out = np.zeros((bs, NQ, C), np.float32)
    for core in range(8):
        out[core // 4] += res.results[core]["outp"].T
    out += inputs["b_out"][None, None, :].astype(np.float32)
    return out
